# revision 31
# baseline (speedup 1.0000x reference)
"""DecoderAttentionSingle Trainium2 Bass kernel (v5).

8 NeuronCores, pure batch-parallel: one [C,H,W] image per core.

Per-core dataflow (bf16 data, fp32 PSUM):
  scores: q+k accumulated directly in PSUM. Host packs fqk [NCH,128,51,W]
      staging rows per chunk: dec rows (16) | encP (18: stacked halves
      [enc w-1 | enc w]) | encQ (17: stacked [enc(y,w+1) | enc(y+1,w+1)]).
      Per 4-row group G and neighbor-group g in {pairs dr=-1,0,+1;
      stacked (dc=+1, dr=-1/0); single (dr=+1,dc=+1)}: matmul wdec2 (q,
      start) + matmul wenc_bd/wenc_lo (k, stop) -> one PSUM bank; ACT
      tanh (bias = b_dec+b_enc) drains pairs of banks -> sp5 planes.
  dots: PE wagg5 matmuls over sp5 planes -> dps [10, 4*W] PSUM; ACT
      drain -> sc_sb (rows 10:15 stay -100).
  XBAR dma transpose sc_sb -> e_pm [128(w), 16, 16(n)] pixel-major.
  softmax: exp (ACT, bias=b_agg), mask/reduce/recip (DVE), normalize into
      x2-duplicated e_pm2 [128, 16, 16, 2].
  MAC on DVE: 9 mults + 8 adds into acc_blk; encoder neighborhoods via
      XBAR transposes from DRAM (w-shift = +-1 element offset; wrap
      garbage masked).
  conv3x3 (PE, 9 matmuls per 4-row window, 1-bank PSUM) -> vals_pc
      parity-packed.
  attn XBAR -> attn_pc [(h%2,c), h2, w] (pre-issued right after MAC);
      out = W2v^T vals + W2a^T attn (block-diag weights); ACT bias drain;
      DVE LeakyReLU; DMA store.
"""

import dataclasses
import sys

sys.path.insert(0, "/opt/trn_rl_repo")

from contextlib import ExitStack

import ml_dtypes
import numpy as np

import concourse.bass as bass
import concourse.mybir as mybir
import concourse.tile as tile
from concourse import bacc
from concourse.bass_utils import run_bass_kernel_spmd

BF16 = mybir.dt.bfloat16
FP32 = mybir.dt.float32
FP8 = mybir.dt.float8e4
AF = mybir.ActivationFunctionType
ALU = mybir.AluOpType

B, ENC, DEC, H, W = 8, 64, 128, 128, 128
N_CORES = 8

OFFS = [(dr, dc) for dr in (-1, 0, 1) for dc in (-1, 0, 1)]
# dps/sc_sb row j holds neighbor NMAP[j]
NMAP = [0, 1, 3, 4, 6, 7, 2, 5, 8]

RC = 16            # rows per chunk
RC2 = 32           # rows per output block
NCH = H // RC      # 8 chunks
ENCF_PAD = 256     # front/back zero pad (elements) of flat enc DRAM image
ENCF_N = 2 * ENCF_PAD + H * W

# fqk staging rows per chunk
FQ_DEC = 0         # dec rows r0..r0+15            (16)
FQ_ENCP = 16       # encP x = r0-1..r0+16          (18)
FQ_ENCQ = 34       # encQ y = r0-1..r0+15          (17)
FQ_NR = 51

SW_QK = 16.0       # fp8 scale on W_dec/W_enc (tanh rescales by 1/SW_QK)
SA_AGG = 64.0      # fp8 scale on W_agg (exp rescales by 1/SA_AGG)

# constq packed offsets (fp8 [128, CONSTQ_N])
OFF_QKPAIR = 0                  # [128, 2, 128] planes (wdec2, wenc_bd)
OFF_QKSING = 256                # [128, 2, 128] planes (wdec, wenc_lo); cols
                                # 64:128 zero so the unused PSUM half is
                                # defined
OFF_AGG01 = 512                 # [128, 2, 16] (cols 10:16 zero pad)
OFF_AGG23 = 544                 # [128, 2, 16]
OFF_AGG4 = 576                  # [64, 16]
CONSTQ_N = 592

# constb packed offsets (bf16 [128, CONSTB_N])
OFF_WENCBD = 0                  # [128, 128] blockdiag(W_enc, W_enc)
OFF_WDEC2 = 128                 # [128, 128] W_dec duplicated cols
OFF_WAGG5 = 256                 # [128, 5*10]
OFF_CONVW = 306                 # [128, 9*64]
OFF_W2V = 882                   # [128, 128] block-diag vals half of W_attn
OFF_W2A = 1010                  # [128, 128] block-diag attn half of W_attn
OFF_WLO = 1138                  # [128, 64] W_enc on bottom rows only
OFF_MASK = 1202                 # [128, 128*16] pixel-major mask
CONSTB_N = OFF_MASK + H * 16


def build_program():
    nc = bacc.Bacc(None, target_bir_lowering=False, debug=False)

    encf_d = nc.dram_tensor("encf", [ENC, ENCF_N], BF16, kind="ExternalInput").ap()
    decp_d = nc.dram_tensor("decp", [DEC, H + 2, W + 2], BF16,
                            kind="ExternalInput").ap()
    fqk_d = nc.dram_tensor("fqk", [NCH, 128, FQ_NR, W], FP8,
                           kind="ExternalInput").ap()
    cq_d = nc.dram_tensor("constq", [128, CONSTQ_N], FP8,
                          kind="ExternalInput").ap()
    cb_d = nc.dram_tensor("constb", [128, CONSTB_N], BF16, kind="ExternalInput").ap()
    cf_d = nc.dram_tensor("constf", [128, 4], FP32, kind="ExternalInput").ap()
    out_d = nc.dram_tensor("out", [ENC, H, W], BF16, kind="ExternalOutput").ap()

    HP, WP = H + 2, W + 2

    with tile.TileContext(nc) as tc, ExitStack() as ctx:
        const = ctx.enter_context(tc.tile_pool(name="const", bufs=1))
        big = ctx.enter_context(tc.tile_pool(name="big", bufs=1))
        fqp = ctx.enter_context(tc.tile_pool(name="fqp", bufs=3))
        sp5p = ctx.enter_context(tc.tile_pool(name="sp5p", bufs=3))
        entp = ctx.enter_context(tc.tile_pool(name="entp", bufs=3))
        catp = ctx.enter_context(tc.tile_pool(name="catp", bufs=2))
        accb = ctx.enter_context(tc.tile_pool(name="accb", bufs=3))
        prodp = ctx.enter_context(tc.tile_pool(name="prodp", bufs=2))
        smal = ctx.enter_context(tc.tile_pool(name="smal", bufs=2))
        outp = ctx.enter_context(tc.tile_pool(name="outp", bufs=1))

        constb = const.tile([128, CONSTB_N], BF16)
        nc.sync.dma_start(constb[:, 0:OFF_MASK], cb_d[:, 0:OFF_MASK])
        nc.sync.dma_start(constb[:, OFF_MASK:], cb_d[:, OFF_MASK:])
        constf = const.tile([128, 4], FP32)
        nc.sync.dma_start(constf[:], cf_d)
        constq = const.tile([128, CONSTQ_N], FP8)
        nc.sync.dma_start(constq[:], cq_d)
        wqk_pair = constq[:, OFF_QKPAIR:OFF_QKPAIR + 256].rearrange(
            "p (t m) -> p t m", t=2)
        wqk_sing = constq[:, OFF_QKSING:OFF_QKSING + 256].rearrange(
            "p (t m) -> p t m", t=2)
        wagg01 = constq[:, OFF_AGG01:OFF_AGG01 + 32].rearrange(
            "p (t m) -> p t m", t=2)
        wagg23 = constq[:, OFF_AGG23:OFF_AGG23 + 32].rearrange(
            "p (t m) -> p t m", t=2)
        wagg4 = constq[0:64, OFF_AGG4:OFF_AGG4 + 16]

        wenc_bd = constb[:, OFF_WENCBD:OFF_WENCBD + 128]
        wdec2 = constb[:, OFF_WDEC2:OFF_WDEC2 + 128]
        wagg5 = constb[:, OFF_WAGG5:OFF_WAGG5 + 50].rearrange(
            "p (g m) -> p g m", g=5)
        convw = constb[:, OFF_CONVW:OFF_CONVW + 576].rearrange(
            "p (n c) -> p n c", n=9)
        w2v = constb[:, OFF_W2V:OFF_W2V + 128]
        w2a = constb[:, OFF_W2A:OFF_W2A + 128]
        wenc_lo = constb[:, OFF_WLO:OFF_WLO + 64]
        maskpm = constb[:, OFF_MASK:OFF_MASK + H * 16].rearrange(
            "p (h n) -> p h n", h=H)
        bsum = constf[:, 0:1]          # b_dec+b_enc, duplicated both halves
        bconv = constf[0:64, 1:2]
        battn2 = constf[:, 2:3]        # b_attn duplicated both halves
        baggb = constf[:, 3:4]         # b_agg replicated on all partitions

        decp = big.tile([DEC, HP, WP], BF16)

        def emit_decp_loads():
            for dli in range(4):
                dr0 = dli * (HP // 4)
                dr1 = HP if dli == 3 else (dli + 1) * (HP // 4)
                nc.sync.dma_start(decp[:, dr0:dr1, :], decp_d[:, dr0:dr1, :])

        # score staging: persistent pair, rows 10:15 stay -100 forever
        sc_sbs = [big.tile([16, RC * W], BF16, tag=f"sc_sb{i}",
                           name=f"sc_sb{i}")
                  for i in range(2)]
        for t in sc_sbs:
            nc.gpsimd.memset(t[:], 0.0)

        psco = ctx.enter_context(
            tc.tile_pool(name="psco", bufs=1, space=bass.MemorySpace.PSUM))
        psd = ctx.enter_context(
            tc.tile_pool(name="psd", bufs=1, space=bass.MemorySpace.PSUM))
        psc = ctx.enter_context(
            tc.tile_pool(name="psc", bufs=1, space=bass.MemorySpace.PSUM))
        psf = ctx.enter_context(
            tc.tile_pool(name="psf", bufs=1, space=bass.MemorySpace.PSUM))

        st = {}   # per-chunk front-stage tiles
        acc_st = {}
        vals_st = {}
        attn_st = {}

        def emit_front_dma(ch):
            r0 = ch * RC
            # fqs staging loads first (scores need them immediately); the
            # encoder neighborhood transposes for the MAC have a chunk of
            # slack.
            fqs = fqp.tile([128, FQ_NR, W], FP8, tag="fqs")
            nc.sync.dma_start(fqs[:, 0:FQ_ENCP, :], fqk_d[ch, :, 0:FQ_ENCP, :])
            nc.sync.dma_start(fqs[:, FQ_ENCP:FQ_ENCQ, :],
                              fqk_d[ch, :, FQ_ENCP:FQ_ENCQ, :])
            nc.sync.dma_start(fqs[:, FQ_ENCQ:, :], fqk_d[ch, :, FQ_ENCQ:, :])
            entv = []
            for vi, dc in enumerate((-1, 0, 1)):
                ev = entp.tile([128, RC + 2, ENC], BF16, tag=f"ent{vi}",
                               name=f"ent{vi}")
                nc.sync.dma_start_transpose(
                    ev[:],
                    encf_d[:, ENCF_PAD + dc + (r0 - 1) * W:
                           ENCF_PAD + dc + (r0 + RC + 1) * W])
                entv.append(ev)
            sp5 = sp5p.tile([128, 5, RC, W], FP8, tag="sp5")
            st[ch] = {"sp5": sp5, "entv": entv, "fqs": fqs}

        def plane2(fqs, rl, delta):
            # [128, 2, 4, W] AP: plane 0 = dec rows rl..rl+3, plane 1 = the
            # rows delta*W elements later (the shifted enc copy).
            a = fqs[:, rl:rl + 4, :].unsqueeze(1)
            ap = [list(d) for d in a.ap]
            ap[1] = [delta * W, 2]
            return dataclasses.replace(a, ap=tuple(tuple(d) for d in ap))

        DBLROW = mybir.MatmulPerfMode.DoubleRow

        def emit_front_g(ch, gi):
            # fused q+k DoubleRow fp8 matmuls into PSUM; tanh((q+k)/SW_QK +
            # bsum) -> sp5 planes, for one 4-row group
            fqs, sp5 = st[ch]["fqs"], st[ch]["sp5"]
            rl = gi * 4
            psA = psco.tile([128, 2, 4, W], FP32, tag="psA")
            for half, dr in enumerate((-1, 0)):
                nc.tensor.matmul(psA[:, half], wqk_pair,
                                 plane2(fqs, rl, FQ_ENCP + 1 + dr),
                                 start=True, stop=True, perf_mode=DBLROW)
            nc.scalar.activation(sp5[:, 0:2, rl:rl + 4, :], psA[:],
                                 AF.Tanh, bias=bsum, scale=1.0 / SW_QK)
            psB = psco.tile([128, 2, 4, W], FP32, tag="psB")
            nc.tensor.matmul(psB[:, 0], wqk_pair,
                             plane2(fqs, rl, FQ_ENCP + 2),
                             start=True, stop=True, perf_mode=DBLROW)
            nc.tensor.matmul(psB[:, 1], wqk_pair,
                             plane2(fqs, rl, FQ_ENCQ),
                             start=True, stop=True, perf_mode=DBLROW)
            nc.scalar.activation(sp5[:, 2:4, rl:rl + 4, :], psB[:],
                                 AF.Tanh, bias=bsum, scale=1.0 / SW_QK)
            psC = psco.tile([128, 4, W], FP32, tag="psC")
            nc.tensor.matmul(psC[:], wqk_sing,
                             plane2(fqs, rl, FQ_ENCQ + 1),
                             start=True, stop=True, perf_mode=DBLROW)
            nc.scalar.activation(sp5[0:64, 4, rl:rl + 4, :], psC[0:64],
                                 AF.Tanh, bias=bsum[0:64], scale=1.0 / SW_QK)

        def emit_attn_tp(b):
            # issue acc -> attn_pc XBAR transpose as soon as block b's MAC is
            # done; finals two chunks later never wait on it.
            acc_blk = acc_st.pop(b)
            attn_pc = catp.tile([128, RC, W], BF16, tag="attn_pc",
                                name="attn_pc")
            nc.sync.dma_start_transpose(
                attn_pc[:], acc_blk[:].rearrange("p h c -> p (h c)"))
            attn_st[b] = attn_pc

        def emit_attn_hc(ch, hc):
            sp5 = st[ch]["sp5"]
            sc_sb = sc_sbs[ch % 2]
            dps = psd.tile([16, 4 * W], FP32, tag="dps")
            rl = hc * 4
            nc.tensor.matmul(dps[:], wagg01, sp5[:, 0:2, rl:rl + 4, :],
                             start=True, stop=False, perf_mode=DBLROW)
            nc.tensor.matmul(dps[:], wagg23, sp5[:, 2:4, rl:rl + 4, :],
                             start=False, stop=False, perf_mode=DBLROW)
            nc.tensor.matmul(
                dps[:], wagg4, sp5[0:64, 4, rl:rl + 4, :],
                start=False, stop=True)
            nc.scalar.activation(
                sc_sb[0:10, hc * 4 * W:(hc + 1) * 4 * W], dps[0:10], AF.Exp,
                bias=baggb[0:10], scale=1.0 / SA_AGG)

        def emit_attn_post(ch):
            r0 = ch * RC
            lr0 = (ch % 2) * RC
            entv = st[ch]["entv"]
            if ch % 2 == 0:
                acc_st[ch // 2] = accb.tile([128, RC2, ENC], BF16,
                                            tag="acc_blk", name="acc_blk")
            acc_blk = acc_st[ch // 2]
            sc_sb = sc_sbs[ch % 2]
            # transpose + softmax
            e_pm = smal.tile([128, RC, 16], BF16, tag="e_pm")
            nc.sync.dma_start_transpose(e_pm[:], sc_sb[:])
            nc.vector.tensor_tensor(
                e_pm[:], e_pm[:], maskpm[:, r0:r0 + RC, :], ALU.mult)
            zs = smal.tile([128, RC], FP32, tag="zs")
            nc.vector.tensor_reduce(
                out=zs[:], in_=e_pm[:], axis=mybir.AxisListType.X, op=ALU.add)
            zr = smal.tile([128, RC], FP32, tag="zr")
            nc.vector.reciprocal(zr[:], zs[:])
            e_pm2 = smal.tile([128, RC, 16, 2], BF16, tag="e_pm2")
            nc.vector.tensor_tensor(
                e_pm2[:],
                e_pm[:].unsqueeze(3).broadcast_to([128, RC, 16, 2]),
                zr[:].unsqueeze(2).unsqueeze(3).broadcast_to([128, RC, 16, 2]),
                ALU.mult)
            # MAC
            a4 = acc_blk.rearrange("p h (a b) -> p h a b", b=2)
            macord = [3] + [j for j in range(9) if j != 3]
            for j in macord:
                dr, dc = OFFS[NMAP[j]]
                src = entv[dc + 1][:, 1 + dr:1 + dr + RC, :].rearrange(
                    "p r (a b) -> p r a b", b=2)
                wsl = e_pm2[:, :, j:j + 1, :].broadcast_to(
                    [128, RC, ENC // 2, 2])
                if j == 3:
                    nc.vector.tensor_tensor(
                        a4[:, lr0:lr0 + RC], wsl, src, ALU.mult)
                else:
                    prod = prodp.tile([128, RC, ENC // 2, 2], BF16,
                                      tag="prod")
                    nc.vector.tensor_tensor(prod[:], wsl, src, ALU.mult)
                    nc.vector.tensor_tensor(
                        a4[:, lr0:lr0 + RC], a4[:, lr0:lr0 + RC], prod[:],
                        ALU.add)

        def emit_conv_w(ch, wi):
            r0 = ch * RC
            if ch % 2 == 0 and wi == 0:
                vals_st[ch // 2] = catp.tile([128, RC, W], BF16,
                                             tag="vals_pc", name="vals_pc")
            vals_pc = vals_st[ch // 2]
            cp = psc.tile([ENC, 4, W], FP32, tag="cp")
            wr0 = r0 + wi * 4
            for n, (dr, dc) in enumerate(OFFS):
                nc.tensor.matmul(
                    cp[:], convw[:, n, :],
                    decp[:, 1 + wr0 + dr:1 + wr0 + 4 + dr,
                         1 + dc:1 + W + dc],
                    start=(n == 0), stop=(n == 8))
            lh2 = ((ch % 2) * RC + wi * 4) // 2
            nc.vector.tensor_copy(
                vals_pc[0:64, lh2:lh2 + 2, :], cp[:, 0::2, :])
            nc.vector.tensor_copy(
                vals_pc[64:128, lh2:lh2 + 2, :], cp[:, 1::2, :])

        def emit_final(b):
            vals_pc = vals_st.pop(b)
            attn_pc = attn_st.pop(b)
            b0 = b * RC2
            outsb = outp.tile([ENC, RC2, W], BF16, tag="outsb")
            for wi in range(RC // 4):
                fp = psf.tile([128, 4, W], FP32, tag="fp")
                fpf = fp[:].rearrange("c r w -> c (r w)")
                nc.tensor.matmul(
                    fpf, w2v, vals_pc[:, wi * 4:(wi + 1) * 4, :],
                    start=True, stop=False)
                nc.tensor.matmul(
                    fpf, w2a, attn_pc[:, wi * 4:(wi + 1) * 4, :],
                    start=False, stop=True)
                ob0 = wi * 8
                tll = smal.tile([128, 4, W], BF16, tag="tll")
                nc.scalar.activation(tll[:], fp[:], AF.Identity, bias=battn2)
                nc.vector.scalar_tensor_tensor(
                    outsb[:, ob0:ob0 + 8:2, :], tll[0:64], 0.2,
                    tll[0:64], ALU.mult, ALU.max)
                nc.vector.scalar_tensor_tensor(
                    outsb[:, ob0 + 1:ob0 + 8:2, :], tll[64:128], 0.2,
                    tll[64:128], ALU.mult, ALU.max)
            nc.sync.dma_start(out_d[:, b0:b0 + RC2, :], outsb[:])

        # Fine-grained interleave: per 4-row group, emit next chunk's
        # q+k+tanh, this chunk's dots, and the conv window from 2 chunks
        # back — so each in-order engine queue alternates independent work
        # and ring-1 PSUM reuse never stalls the head of the queue.
        # Finals fire at chunks 3/5/7; tail is conv6, conv7, final3.
        finals_at = {3: 0, 5: 1, 7: 2}
        emit_front_dma(0)
        emit_front_dma(1)
        emit_decp_loads()
        for gi in range(4):
            emit_front_g(0, gi)
        for gi in range(4):
            emit_front_g(1, gi)
        for ch in range(NCH):
            if ch + 2 < NCH:
                emit_front_dma(ch + 2)
            for gi in range(4):
                emit_attn_hc(ch, gi)
                if ch >= 2:
                    emit_conv_w(ch - 2, gi)
                if ch + 2 < NCH:
                    emit_front_g(ch + 2, gi)
            emit_attn_post(ch)
            if ch % 2 == 1:
                emit_attn_tp(ch // 2)
            if ch in finals_at:
                emit_final(finals_at[ch])
        for wi in range(4):
            emit_conv_w(6, wi)
        for wi in range(4):
            emit_conv_w(7, wi)
        emit_final(3)

    nc.compile()
    return nc


_PROG = None
_RUN_KWARGS = {}
_LAST_RESULT = None


def _get_prog():
    global _PROG
    if _PROG is None:
        _PROG = build_program()
    return _PROG


def _make_mask_pm():
    """[W(part), H, 16] validity mask in NMAP column order."""
    m = np.zeros((W, H, 16), dtype=np.float32)
    for j, n in enumerate(NMAP):
        dr, dc = OFFS[n]
        rv = np.arange(H) + dr
        cv = np.arange(W) + dc
        m[:, :, j] = (((cv >= 0) & (cv < W))[:, None]
                      & ((rv >= 0) & (rv < H))[None, :]).astype(np.float32)
    return m


def _pack_constb(W_dec, W_enc, W_agg, conv_w, W_attn):
    bf = ml_dtypes.bfloat16
    cb = np.zeros((128, CONSTB_N), dtype=np.float32)
    we = np.asarray(W_enc, np.float32)
    cb[0:64, OFF_WENCBD:OFF_WENCBD + 64] = we
    cb[64:128, OFF_WENCBD + 64:OFF_WENCBD + 128] = we
    cb[64:128, OFF_WLO:OFF_WLO + 64] = we
    wd = np.asarray(W_dec, np.float32)
    cb[:, OFF_WDEC2:OFF_WDEC2 + 64] = wd
    cb[:, OFF_WDEC2 + 64:OFF_WDEC2 + 128] = wd
    wa = np.asarray(W_agg, np.float32)[:, 0]
    w5 = np.zeros((128, 5, 10), dtype=np.float32)
    for g in range(4):  # groups 0-2 pairs, group 3 stacked singles
        w5[0:64, g, 2 * g] = wa
        w5[64:128, g, 2 * g + 1] = wa
    w5[0:64, 4, 8] = wa
    cb[:, OFF_WAGG5:OFF_WAGG5 + 50] = w5.reshape(128, 50)
    cw = np.asarray(conv_w, np.float32).reshape(9, DEC, ENC).transpose(1, 0, 2)
    cb[:, OFF_CONVW:OFF_CONVW + 576] = cw.reshape(128, 576)
    wat = np.asarray(W_attn, np.float32)
    for par in range(2):  # block-diag: row parity stays separated
        sl = slice(par * 64, par * 64 + 64)
        cb[sl, OFF_W2V + par * 64:OFF_W2V + par * 64 + 64] = wat[0:64]
        cb[sl, OFF_W2A + par * 64:OFF_W2A + par * 64 + 64] = wat[64:128]
    cb[:, OFF_MASK:OFF_MASK + H * 16] = _make_mask_pm().reshape(128, H * 16)
    return cb.astype(bf)


def _pack_fqk(dec, enc):
    """[NCH, 128, FQ_NR, W] staging: dec rows | encP stacked | encQ stacked.

    dec: [128, H, W]; enc: [64, H, W].
    encP row x: top half enc[c, x, w-1], bottom enc[c, x, w].
    encQ row y: top half enc[c, y, w+1], bottom enc[c, y+1, w+1].
    Out-of-range rows/cols are zero.
    """
    dec = np.asarray(dec, np.float32)
    enc = np.asarray(enc, np.float32)
    encm = np.zeros((ENC, H, W), np.float32)   # enc shifted to w-1 frame
    encm[:, :, 1:] = enc[:, :, :-1]
    encs = np.zeros((ENC, H, W), np.float32)   # enc shifted to w+1 frame
    encs[:, :, :-1] = enc[:, :, 1:]

    def padr(a):  # rows -1 and H become zero; row x lives at index x+1
        return np.pad(a, ((0, 0), (1, 1), (0, 0)))

    encm_p, enc_p, encs_p = padr(encm), padr(enc), padr(encs)
    fqk = np.zeros((NCH, 128, FQ_NR, W), np.float32)
    for ch in range(NCH):
        r0 = ch * RC
        fqk[ch, :, FQ_DEC:FQ_DEC + RC] = dec[:, r0:r0 + RC]
        fqk[ch, 0:64, FQ_ENCP:FQ_ENCQ] = encm_p[:, r0:r0 + RC + 2]
        fqk[ch, 64:128, FQ_ENCP:FQ_ENCQ] = enc_p[:, r0:r0 + RC + 2]
        fqk[ch, 0:64, FQ_ENCQ:FQ_NR] = encs_p[:, r0:r0 + RC + 1]
        fqk[ch, 64:128, FQ_ENCQ:FQ_NR] = encs_p[:, r0 + 1:r0 + RC + 2]
    return fqk.astype(ml_dtypes.float8_e4m3)


def _pack_constq(W_dec, W_enc, W_agg):
    cq = np.zeros((128, CONSTQ_N), np.float32)
    wd = np.asarray(W_dec, np.float32) * SW_QK   # [128, 64]
    we = np.asarray(W_enc, np.float32) * SW_QK   # [64, 64]
    cq[:, OFF_QKPAIR + 0:OFF_QKPAIR + 64] = wd
    cq[:, OFF_QKPAIR + 64:OFF_QKPAIR + 128] = wd
    cq[0:64, OFF_QKPAIR + 128:OFF_QKPAIR + 192] = we
    cq[64:128, OFF_QKPAIR + 192:OFF_QKPAIR + 256] = we
    cq[:, OFF_QKSING:OFF_QKSING + 64] = wd
    cq[64:128, OFF_QKSING + 128:OFF_QKSING + 192] = we
    wa = np.asarray(W_agg, np.float32)[:, 0] * SA_AGG
    w5 = np.zeros((128, 5, 16), np.float32)
    for g in range(4):
        w5[0:64, g, 2 * g] = wa
        w5[64:128, g, 2 * g + 1] = wa
    w5[0:64, 4, 8] = wa
    cq[:, OFF_AGG01:OFF_AGG01 + 16] = w5[:, 0]
    cq[:, OFF_AGG01 + 16:OFF_AGG01 + 32] = w5[:, 1]
    cq[:, OFF_AGG23:OFF_AGG23 + 16] = w5[:, 2]
    cq[:, OFF_AGG23 + 16:OFF_AGG23 + 32] = w5[:, 3]
    cq[0:64, OFF_AGG4:OFF_AGG4 + 16] = w5[0:64, 4]
    return np.clip(cq, -240, 240).astype(ml_dtypes.float8_e4m3)


def kernel(encoder_features, decoder_features, W_enc, b_enc, W_dec, b_dec,
           W_agg, b_agg, W_attn, b_attn, conv_w, conv_b):
    bf = ml_dtypes.bfloat16
    nc = _get_prog()

    cf = np.zeros((128, 4), dtype=np.float32)
    bs = np.asarray(b_dec, np.float32) + np.asarray(b_enc, np.float32)
    cf[0:64, 0] = bs
    cf[64:128, 0] = bs
    battn_eff = (np.asarray(b_attn, np.float32)
                 + np.asarray(conv_b, np.float32)
                 @ np.asarray(W_attn, np.float32)[0:64])
    cf[0:64, 2] = battn_eff
    cf[64:128, 2] = battn_eff
    cf[:, 3] = float(np.asarray(b_agg).reshape(-1)[0])

    shared = {
        "constb": _pack_constb(W_dec, W_enc, W_agg, conv_w, W_attn),
        "constf": cf,
        "constq": _pack_constq(W_dec, W_enc, W_agg),
    }
    enc_all = np.asarray(encoder_features, np.float32).astype(bf)
    dec_all = np.asarray(decoder_features, np.float32).astype(bf)
    in_maps = []
    for c in range(N_CORES):
        encf = np.zeros((ENC, ENCF_N), dtype=bf)
        encf[:, ENCF_PAD:ENCF_PAD + H * W] = enc_all[c].reshape(ENC, H * W)
        decp = np.zeros((DEC, H + 2, W + 2), dtype=bf)
        decp[:, 1:H + 1, 1:W + 1] = dec_all[c]
        m = dict(shared)
        m["encf"] = encf
        m["decp"] = decp
        m["fqk"] = _pack_fqk(dec_all[c].astype(np.float32),
                             enc_all[c].astype(np.float32))
        in_maps.append(m)

    res = run_bass_kernel_spmd(nc, in_maps, list(range(N_CORES)),
                               **_RUN_KWARGS)
    global _LAST_RESULT
    _LAST_RESULT = res
    out = np.stack(
        [np.asarray(res.results[c]["out"], np.float32) for c in range(N_CORES)])
    return out


# revision 32
# speedup vs baseline: 1.0985x; 1.0985x over previous
"""DecoderAttentionSingle Trainium2 Bass kernel (v5).

8 NeuronCores, pure batch-parallel: one [C,H,W] image per core.

Per-core dataflow (bf16 data, fp32 PSUM):
  scores: q+k accumulated directly in PSUM. Host packs fqk [NCH,128,51,W]
      staging rows per chunk: dec rows (16) | encP (18: stacked halves
      [enc w-1 | enc w]) | encQ (17: stacked [enc(y,w+1) | enc(y+1,w+1)]).
      Per 4-row group G and neighbor-group g in {pairs dr=-1,0,+1;
      stacked (dc=+1, dr=-1/0); single (dr=+1,dc=+1)}: matmul wdec2 (q,
      start) + matmul wenc_bd/wenc_lo (k, stop) -> one PSUM bank; ACT
      tanh (bias = b_dec+b_enc) drains pairs of banks -> sp5 planes.
  dots: PE wagg5 matmuls over sp5 planes -> dps [10, 4*W] PSUM; ACT
      drain -> sc_sb (rows 10:15 stay -100).
  XBAR dma transpose sc_sb -> e_pm [128(w), 16, 16(n)] pixel-major.
  softmax: exp (ACT, bias=b_agg), mask/reduce/recip (DVE), normalize into
      x2-duplicated e_pm2 [128, 16, 16, 2].
  MAC on DVE: 9 mults + 8 adds into acc_blk; encoder neighborhoods via
      XBAR transposes from DRAM (w-shift = +-1 element offset; wrap
      garbage masked).
  conv3x3 (PE, 9 matmuls per 4-row window, 1-bank PSUM) -> vals_pc
      parity-packed.
  attn XBAR -> attn_pc [(h%2,c), h2, w] (pre-issued right after MAC);
      out = W2v^T vals + W2a^T attn (block-diag weights); ACT bias drain;
      DVE LeakyReLU; DMA store.
"""

import dataclasses
import sys

sys.path.insert(0, "/opt/trn_rl_repo")

from contextlib import ExitStack

import ml_dtypes
import numpy as np

import concourse.bass as bass
import concourse.mybir as mybir
import concourse.tile as tile
from concourse import bacc
from concourse.bass_utils import run_bass_kernel_spmd

BF16 = mybir.dt.bfloat16
FP32 = mybir.dt.float32
FP8 = mybir.dt.float8e4
AF = mybir.ActivationFunctionType
ALU = mybir.AluOpType

B, ENC, DEC, H, W = 8, 64, 128, 128, 128
N_CORES = 8

OFFS = [(dr, dc) for dr in (-1, 0, 1) for dc in (-1, 0, 1)]
# dps/sc_sb row j holds neighbor NMAP[j]
NMAP = [0, 1, 3, 4, 6, 7, 2, 5, 8]

RC = 16            # rows per chunk
RC2 = 32           # rows per output block
NCH = H // RC      # 8 chunks
ENCF_PAD = 256     # front/back zero pad (elements) of flat enc DRAM image
ENCF_N = 2 * ENCF_PAD + H * W

# fqk staging rows per chunk
FQ_DEC = 0         # dec rows r0..r0+15            (16)
FQ_ENCP = 16       # encP x = r0-1..r0+16          (18)
FQ_ENCQ = 34       # encQ y = r0-1..r0+15          (17)
FQ_NR = 51

SW_QK = 16.0       # fp8 scale on W_dec/W_enc (tanh rescales by 1/SW_QK)
SA_AGG = 64.0      # fp8 scale on W_agg (exp rescales by 1/SA_AGG)

# constq packed offsets (fp8 [128, CONSTQ_N])
OFF_QKPAIR = 0                  # [128, 2, 128] planes (wdec2, wenc_bd)
OFF_QKSING = 256                # [128, 2, 128] planes (wdec, wenc_lo); cols
                                # 64:128 zero so the unused PSUM half is
                                # defined
OFF_AGG01 = 512                 # [128, 2, 16] (cols 10:16 zero pad)
OFF_AGG23 = 544                 # [128, 2, 16]
OFF_AGG4 = 576                  # [64, 16]
CONSTQ_N = 592

# constb packed offsets (bf16 [128, CONSTB_N])
OFF_WENCBD = 0                  # [128, 128] blockdiag(W_enc, W_enc)
OFF_WDEC2 = 128                 # [128, 128] W_dec duplicated cols
OFF_WAGG5 = 256                 # [128, 5*10]
OFF_CONVW = 306                 # [128, 9*64]
OFF_W2V = 882                   # [128, 128] block-diag vals half of W_attn
OFF_W2A = 1010                  # [128, 128] block-diag attn half of W_attn
OFF_WLO = 1138                  # [128, 64] W_enc on bottom rows only
OFF_MASK = 1202                 # [128, 128*16] pixel-major mask
CONSTB_N = OFF_MASK + H * 16


def build_program():
    nc = bacc.Bacc(None, target_bir_lowering=False, debug=False)

    encf_d = nc.dram_tensor("encf", [ENC, ENCF_N], BF16, kind="ExternalInput").ap()
    decp_d = nc.dram_tensor("decp", [DEC, H + 2, W + 2], BF16,
                            kind="ExternalInput").ap()
    fqk_d = nc.dram_tensor("fqk", [NCH, 128, FQ_NR, W], FP8,
                           kind="ExternalInput").ap()
    cq_d = nc.dram_tensor("constq", [128, CONSTQ_N], FP8,
                          kind="ExternalInput").ap()
    cb_d = nc.dram_tensor("constb", [128, CONSTB_N], BF16, kind="ExternalInput").ap()
    cf_d = nc.dram_tensor("constf", [128, 4], FP32, kind="ExternalInput").ap()
    out_d = nc.dram_tensor("out", [ENC, H, W], BF16, kind="ExternalOutput").ap()

    HP, WP = H + 2, W + 2

    with tile.TileContext(nc) as tc, ExitStack() as ctx:
        const = ctx.enter_context(tc.tile_pool(name="const", bufs=1))
        big = ctx.enter_context(tc.tile_pool(name="big", bufs=1))
        fqp = ctx.enter_context(tc.tile_pool(name="fqp", bufs=3))
        sp5p = ctx.enter_context(tc.tile_pool(name="sp5p", bufs=3))
        entp = ctx.enter_context(tc.tile_pool(name="entp", bufs=3))
        catp = ctx.enter_context(tc.tile_pool(name="catp", bufs=2))
        accb = ctx.enter_context(tc.tile_pool(name="accb", bufs=3))
        prodp = ctx.enter_context(tc.tile_pool(name="prodp", bufs=2))
        smal = ctx.enter_context(tc.tile_pool(name="smal", bufs=2))
        outp = ctx.enter_context(tc.tile_pool(name="outp", bufs=1))

        constb = const.tile([128, CONSTB_N], BF16)
        nc.sync.dma_start(constb[:, 0:OFF_MASK], cb_d[:, 0:OFF_MASK])
        nc.sync.dma_start(constb[:, OFF_MASK:], cb_d[:, OFF_MASK:])
        constf = const.tile([128, 4], FP32)
        nc.sync.dma_start(constf[:], cf_d)
        constq = const.tile([128, CONSTQ_N], FP8)
        nc.sync.dma_start(constq[:], cq_d)
        wqk_pair = constq[:, OFF_QKPAIR:OFF_QKPAIR + 256].rearrange(
            "p (t m) -> p t m", t=2)
        wqk_sing = constq[:, OFF_QKSING:OFF_QKSING + 256].rearrange(
            "p (t m) -> p t m", t=2)
        wagg01 = constq[:, OFF_AGG01:OFF_AGG01 + 32].rearrange(
            "p (t m) -> p t m", t=2)
        wagg23 = constq[:, OFF_AGG23:OFF_AGG23 + 32].rearrange(
            "p (t m) -> p t m", t=2)
        wagg4 = constq[0:64, OFF_AGG4:OFF_AGG4 + 16]

        wenc_bd = constb[:, OFF_WENCBD:OFF_WENCBD + 128]
        wdec2 = constb[:, OFF_WDEC2:OFF_WDEC2 + 128]
        wagg5 = constb[:, OFF_WAGG5:OFF_WAGG5 + 50].rearrange(
            "p (g m) -> p g m", g=5)
        convw = constb[:, OFF_CONVW:OFF_CONVW + 576].rearrange(
            "p (n c) -> p n c", n=9)
        w2v = constb[:, OFF_W2V:OFF_W2V + 128]
        w2a = constb[:, OFF_W2A:OFF_W2A + 128]
        wenc_lo = constb[:, OFF_WLO:OFF_WLO + 64]
        maskpm = constb[:, OFF_MASK:OFF_MASK + H * 16].rearrange(
            "p (h n) -> p h n", h=H)
        bsum = constf[:, 0:1]          # b_dec+b_enc, duplicated both halves
        bconv = constf[0:64, 1:2]
        battn2 = constf[:, 2:3]        # b_attn duplicated both halves
        baggb = constf[:, 3:4]         # b_agg replicated on all partitions

        decp = big.tile([DEC, HP, WP], BF16)

        def emit_decp_loads():
            for dli in range(4):
                dr0 = dli * (HP // 4)
                dr1 = HP if dli == 3 else (dli + 1) * (HP // 4)
                nc.sync.dma_start(decp[:, dr0:dr1, :], decp_d[:, dr0:dr1, :])

        # score staging: persistent pair, rows 10:15 stay -100 forever
        sc_sbs = [big.tile([16, RC * W], BF16, tag=f"sc_sb{i}",
                           name=f"sc_sb{i}")
                  for i in range(2)]
        for t in sc_sbs:
            nc.gpsimd.memset(t[:], 0.0)

        psco = ctx.enter_context(
            tc.tile_pool(name="psco", bufs=1, space=bass.MemorySpace.PSUM))
        psd = ctx.enter_context(
            tc.tile_pool(name="psd", bufs=1, space=bass.MemorySpace.PSUM))
        psc = ctx.enter_context(
            tc.tile_pool(name="psc", bufs=1, space=bass.MemorySpace.PSUM))
        psf = ctx.enter_context(
            tc.tile_pool(name="psf", bufs=1, space=bass.MemorySpace.PSUM))

        st = {}   # per-chunk front-stage tiles
        acc_st = {}
        vals_st = {}
        attn_st = {}

        def emit_front_dma(ch):
            r0 = ch * RC
            # fqs staging loads first (scores need them immediately); the
            # encoder neighborhood transposes for the MAC have a chunk of
            # slack.
            fqs = fqp.tile([128, FQ_NR, W], FP8, tag="fqs")
            nc.sync.dma_start(fqs[:, 0:FQ_ENCP, :], fqk_d[ch, :, 0:FQ_ENCP, :])
            nc.sync.dma_start(fqs[:, FQ_ENCP:FQ_ENCQ, :],
                              fqk_d[ch, :, FQ_ENCP:FQ_ENCQ, :])
            nc.sync.dma_start(fqs[:, FQ_ENCQ:, :], fqk_d[ch, :, FQ_ENCQ:, :])
            entv = []
            for vi, dc in enumerate((-1, 0, 1)):
                ev = entp.tile([128, RC + 2, ENC], BF16, tag=f"ent{vi}",
                               name=f"ent{vi}")
                nc.sync.dma_start_transpose(
                    ev[:],
                    encf_d[:, ENCF_PAD + dc + (r0 - 1) * W:
                           ENCF_PAD + dc + (r0 + RC + 1) * W])
                entv.append(ev)
            sp5 = sp5p.tile([128, 5, RC, W], FP8, tag="sp5")
            st[ch] = {"sp5": sp5, "entv": entv, "fqs": fqs}

        def plane2(fqs, rl, delta):
            # [128, 2, 4, W] AP: plane 0 = dec rows rl..rl+3, plane 1 = the
            # rows delta*W elements later (the shifted enc copy).
            a = fqs[:, rl:rl + 4, :].unsqueeze(1)
            ap = [list(d) for d in a.ap]
            ap[1] = [delta * W, 2]
            return dataclasses.replace(a, ap=tuple(tuple(d) for d in ap))

        DBLROW = mybir.MatmulPerfMode.DoubleRow

        def emit_front_g(ch, gi):
            # fused q+k DoubleRow fp8 matmuls into PSUM; tanh((q+k)/SW_QK +
            # bsum) -> sp5 planes, for one 4-row group
            fqs, sp5 = st[ch]["fqs"], st[ch]["sp5"]
            rl = gi * 4
            psA = psco.tile([128, 2, 4, W], FP32, tag="psA")
            for half, dr in enumerate((-1, 0)):
                nc.tensor.matmul(psA[:, half], wqk_pair,
                                 plane2(fqs, rl, FQ_ENCP + 1 + dr),
                                 start=True, stop=True, perf_mode=DBLROW)
            nc.scalar.activation(sp5[:, 0:2, rl:rl + 4, :], psA[:],
                                 AF.Tanh, bias=bsum, scale=1.0 / SW_QK)
            psB = psco.tile([128, 2, 4, W], FP32, tag="psB")
            nc.tensor.matmul(psB[:, 0], wqk_pair,
                             plane2(fqs, rl, FQ_ENCP + 2),
                             start=True, stop=True, perf_mode=DBLROW)
            nc.tensor.matmul(psB[:, 1], wqk_pair,
                             plane2(fqs, rl, FQ_ENCQ),
                             start=True, stop=True, perf_mode=DBLROW)
            nc.scalar.activation(sp5[:, 2:4, rl:rl + 4, :], psB[:],
                                 AF.Tanh, bias=bsum, scale=1.0 / SW_QK)
            psC = psco.tile([128, 4, W], FP32, tag="psC")
            nc.tensor.matmul(psC[:], wqk_sing,
                             plane2(fqs, rl, FQ_ENCQ + 1),
                             start=True, stop=True, perf_mode=DBLROW)
            nc.scalar.activation(sp5[0:64, 4, rl:rl + 4, :], psC[0:64],
                                 AF.Tanh, bias=bsum[0:64], scale=1.0 / SW_QK)

        def emit_attn_tp(b):
            # issue acc -> attn_pc XBAR transpose as soon as block b's MAC is
            # done; finals two chunks later never wait on it.
            acc_blk = acc_st.pop(b)
            attn_pc = catp.tile([128, RC, W], BF16, tag="attn_pc",
                                name="attn_pc")
            nc.sync.dma_start_transpose(
                attn_pc[:], acc_blk[:].rearrange("p h c -> p (h c)"))
            attn_st[b] = attn_pc

        def emit_attn_hc(ch, hc):
            sp5 = st[ch]["sp5"]
            sc_sb = sc_sbs[ch % 2]
            dps = psd.tile([16, 4 * W], FP32, tag="dps")
            rl = hc * 4
            nc.tensor.matmul(dps[:], wagg01, sp5[:, 0:2, rl:rl + 4, :],
                             start=True, stop=False, perf_mode=DBLROW)
            nc.tensor.matmul(dps[:], wagg23, sp5[:, 2:4, rl:rl + 4, :],
                             start=False, stop=False, perf_mode=DBLROW)
            nc.tensor.matmul(
                dps[:], wagg4, sp5[0:64, 4, rl:rl + 4, :],
                start=False, stop=True)
            nc.scalar.activation(
                sc_sb[0:10, hc * 4 * W:(hc + 1) * 4 * W], dps[0:10], AF.Exp,
                bias=baggb[0:10], scale=1.0 / SA_AGG)

        def emit_attn_post(ch):
            r0 = ch * RC
            lr0 = (ch % 2) * RC
            entv = st[ch]["entv"]
            if ch % 2 == 0:
                acc_st[ch // 2] = accb.tile([128, RC2, ENC], BF16,
                                            tag="acc_blk", name="acc_blk")
            acc_blk = acc_st[ch // 2]
            sc_sb = sc_sbs[ch % 2]
            # transpose + softmax
            e_pm = smal.tile([128, RC, 16], BF16, tag="e_pm")
            nc.sync.dma_start_transpose(e_pm[:], sc_sb[:])
            nc.vector.tensor_tensor(
                e_pm[:], e_pm[:], maskpm[:, r0:r0 + RC, :], ALU.mult)
            zs = smal.tile([128, RC], FP32, tag="zs")
            nc.vector.tensor_reduce(
                out=zs[:], in_=e_pm[:], axis=mybir.AxisListType.X, op=ALU.add)
            zr = smal.tile([128, RC], FP32, tag="zr")
            nc.vector.reciprocal(zr[:], zs[:])
            e_pm2 = smal.tile([128, RC, 16, 2], BF16, tag="e_pm2")
            nc.vector.tensor_tensor(
                e_pm2[:],
                e_pm[:].unsqueeze(3).broadcast_to([128, RC, 16, 2]),
                zr[:].unsqueeze(2).unsqueeze(3).broadcast_to([128, RC, 16, 2]),
                ALU.mult)
            # MAC
            a4 = acc_blk.rearrange("p h (a b) -> p h a b", b=2)
            macord = [3] + [j for j in range(9) if j != 3]
            for j in macord:
                dr, dc = OFFS[NMAP[j]]
                src = entv[dc + 1][:, 1 + dr:1 + dr + RC, :].rearrange(
                    "p r (a b) -> p r a b", b=2)
                wsl = e_pm2[:, :, j:j + 1, :].broadcast_to(
                    [128, RC, ENC // 2, 2])
                if j == 3:
                    nc.vector.tensor_tensor(
                        a4[:, lr0:lr0 + RC], wsl, src, ALU.mult)
                else:
                    prod = prodp.tile([128, RC, ENC // 2, 2], BF16,
                                      tag="prod")
                    nc.vector.tensor_tensor(prod[:], wsl, src, ALU.mult)
                    nc.vector.tensor_tensor(
                        a4[:, lr0:lr0 + RC], a4[:, lr0:lr0 + RC], prod[:],
                        ALU.add)

        def emit_conv_w(ch, wi):
            r0 = ch * RC
            if ch % 2 == 0 and wi == 0:
                vals_st[ch // 2] = catp.tile([128, RC, W], BF16,
                                             tag="vals_pc", name="vals_pc")
            vals_pc = vals_st[ch // 2]
            cp = psc.tile([ENC, 4, W], FP32, tag="cp")
            wr0 = r0 + wi * 4
            for n, (dr, dc) in enumerate(OFFS):
                nc.tensor.matmul(
                    cp[:], convw[:, n, :],
                    decp[:, 1 + wr0 + dr:1 + wr0 + 4 + dr,
                         1 + dc:1 + W + dc],
                    start=(n == 0), stop=(n == 8))
            lh2 = ((ch % 2) * RC + wi * 4) // 2
            nc.scalar.activation(
                vals_pc[0:64, lh2:lh2 + 2, :], cp[:, 0::2, :], AF.Copy)
            nc.scalar.activation(
                vals_pc[64:128, lh2:lh2 + 2, :], cp[:, 1::2, :], AF.Copy)

        def emit_final(b):
            vals_pc = vals_st.pop(b)
            attn_pc = attn_st.pop(b)
            b0 = b * RC2
            outsb = outp.tile([ENC, RC2, W], BF16, tag="outsb")
            for wi in range(RC // 4):
                fp = psf.tile([128, 4, W], FP32, tag="fp")
                fpf = fp[:].rearrange("c r w -> c (r w)")
                nc.tensor.matmul(
                    fpf, w2v, vals_pc[:, wi * 4:(wi + 1) * 4, :],
                    start=True, stop=False)
                nc.tensor.matmul(
                    fpf, w2a, attn_pc[:, wi * 4:(wi + 1) * 4, :],
                    start=False, stop=True)
                ob0 = wi * 8
                tll = smal.tile([128, 4, W], BF16, tag="tll")
                nc.scalar.activation(tll[:], fp[:], AF.Identity, bias=battn2)
                nc.vector.scalar_tensor_tensor(
                    outsb[:, ob0:ob0 + 8:2, :], tll[0:64], 0.2,
                    tll[0:64], ALU.mult, ALU.max)
                nc.vector.scalar_tensor_tensor(
                    outsb[:, ob0 + 1:ob0 + 8:2, :], tll[64:128], 0.2,
                    tll[64:128], ALU.mult, ALU.max)
            nc.sync.dma_start(out_d[:, b0:b0 + RC2, :], outsb[:])

        # Fine-grained interleave: per 4-row group, emit next chunk's
        # q+k+tanh, this chunk's dots, and the conv window from 2 chunks
        # back — so each in-order engine queue alternates independent work
        # and ring-1 PSUM reuse never stalls the head of the queue.
        # Finals fire at chunks 3/5/7; tail is conv6, conv7, final3.
        finals_at = {3: 0, 5: 1, 7: 2}
        emit_front_dma(0)
        emit_front_dma(1)
        emit_decp_loads()
        for gi in range(4):
            emit_front_g(0, gi)
        for gi in range(4):
            emit_front_g(1, gi)
        for ch in range(NCH):
            if ch + 2 < NCH:
                emit_front_dma(ch + 2)
            for gi in range(4):
                emit_attn_hc(ch, gi)
                if ch >= 2:
                    emit_conv_w(ch - 2, gi)
                if ch + 2 < NCH:
                    emit_front_g(ch + 2, gi)
            emit_attn_post(ch)
            if ch % 2 == 1:
                emit_attn_tp(ch // 2)
            if ch in finals_at:
                emit_final(finals_at[ch])
        for wi in range(4):
            emit_conv_w(6, wi)
        for wi in range(4):
            emit_conv_w(7, wi)
        emit_final(3)

    nc.compile()
    return nc


_PROG = None
_RUN_KWARGS = {}
_LAST_RESULT = None


def _get_prog():
    global _PROG
    if _PROG is None:
        _PROG = build_program()
    return _PROG


def _make_mask_pm():
    """[W(part), H, 16] validity mask in NMAP column order."""
    m = np.zeros((W, H, 16), dtype=np.float32)
    for j, n in enumerate(NMAP):
        dr, dc = OFFS[n]
        rv = np.arange(H) + dr
        cv = np.arange(W) + dc
        m[:, :, j] = (((cv >= 0) & (cv < W))[:, None]
                      & ((rv >= 0) & (rv < H))[None, :]).astype(np.float32)
    return m


def _pack_constb(W_dec, W_enc, W_agg, conv_w, W_attn):
    bf = ml_dtypes.bfloat16
    cb = np.zeros((128, CONSTB_N), dtype=np.float32)
    we = np.asarray(W_enc, np.float32)
    cb[0:64, OFF_WENCBD:OFF_WENCBD + 64] = we
    cb[64:128, OFF_WENCBD + 64:OFF_WENCBD + 128] = we
    cb[64:128, OFF_WLO:OFF_WLO + 64] = we
    wd = np.asarray(W_dec, np.float32)
    cb[:, OFF_WDEC2:OFF_WDEC2 + 64] = wd
    cb[:, OFF_WDEC2 + 64:OFF_WDEC2 + 128] = wd
    wa = np.asarray(W_agg, np.float32)[:, 0]
    w5 = np.zeros((128, 5, 10), dtype=np.float32)
    for g in range(4):  # groups 0-2 pairs, group 3 stacked singles
        w5[0:64, g, 2 * g] = wa
        w5[64:128, g, 2 * g + 1] = wa
    w5[0:64, 4, 8] = wa
    cb[:, OFF_WAGG5:OFF_WAGG5 + 50] = w5.reshape(128, 50)
    cw = np.asarray(conv_w, np.float32).reshape(9, DEC, ENC).transpose(1, 0, 2)
    cb[:, OFF_CONVW:OFF_CONVW + 576] = cw.reshape(128, 576)
    wat = np.asarray(W_attn, np.float32)
    for par in range(2):  # block-diag: row parity stays separated
        sl = slice(par * 64, par * 64 + 64)
        cb[sl, OFF_W2V + par * 64:OFF_W2V + par * 64 + 64] = wat[0:64]
        cb[sl, OFF_W2A + par * 64:OFF_W2A + par * 64 + 64] = wat[64:128]
    cb[:, OFF_MASK:OFF_MASK + H * 16] = _make_mask_pm().reshape(128, H * 16)
    return cb.astype(bf)


def _pack_fqk(dec, enc):
    """[NCH, 128, FQ_NR, W] staging: dec rows | encP stacked | encQ stacked.

    dec: [128, H, W]; enc: [64, H, W].
    encP row x: top half enc[c, x, w-1], bottom enc[c, x, w].
    encQ row y: top half enc[c, y, w+1], bottom enc[c, y+1, w+1].
    Out-of-range rows/cols are zero.
    """
    dec = np.asarray(dec, np.float32)
    enc = np.asarray(enc, np.float32)
    encm = np.zeros((ENC, H, W), np.float32)   # enc shifted to w-1 frame
    encm[:, :, 1:] = enc[:, :, :-1]
    encs = np.zeros((ENC, H, W), np.float32)   # enc shifted to w+1 frame
    encs[:, :, :-1] = enc[:, :, 1:]

    def padr(a):  # rows -1 and H become zero; row x lives at index x+1
        return np.pad(a, ((0, 0), (1, 1), (0, 0)))

    encm_p, enc_p, encs_p = padr(encm), padr(enc), padr(encs)
    fqk = np.zeros((NCH, 128, FQ_NR, W), np.float32)
    for ch in range(NCH):
        r0 = ch * RC
        fqk[ch, :, FQ_DEC:FQ_DEC + RC] = dec[:, r0:r0 + RC]
        fqk[ch, 0:64, FQ_ENCP:FQ_ENCQ] = encm_p[:, r0:r0 + RC + 2]
        fqk[ch, 64:128, FQ_ENCP:FQ_ENCQ] = enc_p[:, r0:r0 + RC + 2]
        fqk[ch, 0:64, FQ_ENCQ:FQ_NR] = encs_p[:, r0:r0 + RC + 1]
        fqk[ch, 64:128, FQ_ENCQ:FQ_NR] = encs_p[:, r0 + 1:r0 + RC + 2]
    return fqk.astype(ml_dtypes.float8_e4m3)


def _pack_constq(W_dec, W_enc, W_agg):
    cq = np.zeros((128, CONSTQ_N), np.float32)
    wd = np.asarray(W_dec, np.float32) * SW_QK   # [128, 64]
    we = np.asarray(W_enc, np.float32) * SW_QK   # [64, 64]
    cq[:, OFF_QKPAIR + 0:OFF_QKPAIR + 64] = wd
    cq[:, OFF_QKPAIR + 64:OFF_QKPAIR + 128] = wd
    cq[0:64, OFF_QKPAIR + 128:OFF_QKPAIR + 192] = we
    cq[64:128, OFF_QKPAIR + 192:OFF_QKPAIR + 256] = we
    cq[:, OFF_QKSING:OFF_QKSING + 64] = wd
    cq[64:128, OFF_QKSING + 128:OFF_QKSING + 192] = we
    wa = np.asarray(W_agg, np.float32)[:, 0] * SA_AGG
    w5 = np.zeros((128, 5, 16), np.float32)
    for g in range(4):
        w5[0:64, g, 2 * g] = wa
        w5[64:128, g, 2 * g + 1] = wa
    w5[0:64, 4, 8] = wa
    cq[:, OFF_AGG01:OFF_AGG01 + 16] = w5[:, 0]
    cq[:, OFF_AGG01 + 16:OFF_AGG01 + 32] = w5[:, 1]
    cq[:, OFF_AGG23:OFF_AGG23 + 16] = w5[:, 2]
    cq[:, OFF_AGG23 + 16:OFF_AGG23 + 32] = w5[:, 3]
    cq[0:64, OFF_AGG4:OFF_AGG4 + 16] = w5[0:64, 4]
    return np.clip(cq, -240, 240).astype(ml_dtypes.float8_e4m3)


def kernel(encoder_features, decoder_features, W_enc, b_enc, W_dec, b_dec,
           W_agg, b_agg, W_attn, b_attn, conv_w, conv_b):
    bf = ml_dtypes.bfloat16
    nc = _get_prog()

    cf = np.zeros((128, 4), dtype=np.float32)
    bs = np.asarray(b_dec, np.float32) + np.asarray(b_enc, np.float32)
    cf[0:64, 0] = bs
    cf[64:128, 0] = bs
    battn_eff = (np.asarray(b_attn, np.float32)
                 + np.asarray(conv_b, np.float32)
                 @ np.asarray(W_attn, np.float32)[0:64])
    cf[0:64, 2] = battn_eff
    cf[64:128, 2] = battn_eff
    cf[:, 3] = float(np.asarray(b_agg).reshape(-1)[0])

    shared = {
        "constb": _pack_constb(W_dec, W_enc, W_agg, conv_w, W_attn),
        "constf": cf,
        "constq": _pack_constq(W_dec, W_enc, W_agg),
    }
    enc_all = np.asarray(encoder_features, np.float32).astype(bf)
    dec_all = np.asarray(decoder_features, np.float32).astype(bf)
    in_maps = []
    for c in range(N_CORES):
        encf = np.zeros((ENC, ENCF_N), dtype=bf)
        encf[:, ENCF_PAD:ENCF_PAD + H * W] = enc_all[c].reshape(ENC, H * W)
        decp = np.zeros((DEC, H + 2, W + 2), dtype=bf)
        decp[:, 1:H + 1, 1:W + 1] = dec_all[c]
        m = dict(shared)
        m["encf"] = encf
        m["decp"] = decp
        m["fqk"] = _pack_fqk(dec_all[c].astype(np.float32),
                             enc_all[c].astype(np.float32))
        in_maps.append(m)

    res = run_bass_kernel_spmd(nc, in_maps, list(range(N_CORES)),
                               **_RUN_KWARGS)
    global _LAST_RESULT
    _LAST_RESULT = res
    out = np.stack(
        [np.asarray(res.results[c]["out"], np.float32) for c in range(N_CORES)])
    return out


# revision 33
# speedup vs baseline: 1.1174x; 1.0173x over previous
"""DecoderAttentionSingle Trainium2 Bass kernel (v5).

8 NeuronCores, pure batch-parallel: one [C,H,W] image per core.

Per-core dataflow (bf16 data, fp32 PSUM):
  scores: q+k accumulated directly in PSUM. Host packs fqk [NCH,128,51,W]
      staging rows per chunk: dec rows (16) | encP (18: stacked halves
      [enc w-1 | enc w]) | encQ (17: stacked [enc(y,w+1) | enc(y+1,w+1)]).
      Per 4-row group G and neighbor-group g in {pairs dr=-1,0,+1;
      stacked (dc=+1, dr=-1/0); single (dr=+1,dc=+1)}: matmul wdec2 (q,
      start) + matmul wenc_bd/wenc_lo (k, stop) -> one PSUM bank; ACT
      tanh (bias = b_dec+b_enc) drains pairs of banks -> sp5 planes.
  dots: PE wagg5 matmuls over sp5 planes -> dps [10, 4*W] PSUM; ACT
      drain -> sc_sb (rows 10:15 stay -100).
  XBAR dma transpose sc_sb -> e_pm [128(w), 16, 16(n)] pixel-major.
  softmax: exp (ACT, bias=b_agg), mask/reduce/recip (DVE), normalize into
      x2-duplicated e_pm2 [128, 16, 16, 2].
  MAC on DVE: 9 mults + 8 adds into acc_blk; encoder neighborhoods via
      XBAR transposes from DRAM (w-shift = +-1 element offset; wrap
      garbage masked).
  conv3x3 (PE, 9 matmuls per 4-row window, 1-bank PSUM) -> vals_pc
      parity-packed.
  attn XBAR -> attn_pc [(h%2,c), h2, w] (pre-issued right after MAC);
      out = W2v^T vals + W2a^T attn (block-diag weights); ACT bias drain;
      DVE LeakyReLU; DMA store.
"""

import dataclasses
import sys

sys.path.insert(0, "/opt/trn_rl_repo")

from contextlib import ExitStack

import ml_dtypes
import numpy as np

import concourse.bass as bass
import concourse.mybir as mybir
import concourse.tile as tile
from concourse import bacc
from concourse.bass_utils import run_bass_kernel_spmd

BF16 = mybir.dt.bfloat16
FP32 = mybir.dt.float32
FP8 = mybir.dt.float8e4
AF = mybir.ActivationFunctionType
ALU = mybir.AluOpType

B, ENC, DEC, H, W = 8, 64, 128, 128, 128
N_CORES = 8

OFFS = [(dr, dc) for dr in (-1, 0, 1) for dc in (-1, 0, 1)]
# dps/sc_sb row j holds neighbor NMAP[j]
NMAP = [0, 1, 3, 4, 6, 7, 2, 5, 8]

RC = 16            # rows per chunk
RC2 = 32           # rows per output block
NCH = H // RC      # 8 chunks
ENCF_PAD = 256     # front/back zero pad (elements) of flat enc DRAM image
ENCF_N = 2 * ENCF_PAD + H * W

# fqk staging rows per chunk
FQ_DEC = 0         # dec rows r0..r0+15            (16)
FQ_ENCP = 16       # encP x = r0-1..r0+16          (18)
FQ_ENCQ = 34       # encQ y = r0-1..r0+15          (17)
FQ_NR = 51

SW_QK = 16.0       # fp8 scale on W_dec/W_enc (tanh rescales by 1/SW_QK)
SA_AGG = 64.0      # fp8 scale on W_agg (exp rescales by 1/SA_AGG)

# constq packed offsets (fp8 [128, CONSTQ_N])
OFF_QKPAIR = 0                  # [128, 2, 128] planes (wdec2, wenc_bd)
OFF_QKSING = 256                # [128, 2, 64] planes (wdec, wenc_lo)
OFF_AGG01 = 384                 # [128, 2, 16] (cols 10:16 zero pad)
OFF_AGG23 = 416                 # [128, 2, 16]
OFF_AGG4 = 448                  # [64, 16]
CONSTQ_N = 464

# constb packed offsets (bf16 [128, CONSTB_N])
OFF_WENCBD = 0                  # [128, 128] blockdiag(W_enc, W_enc)
OFF_WDEC2 = 128                 # [128, 128] W_dec duplicated cols
OFF_WAGG5 = 256                 # [128, 5*10]
OFF_CONVW = 306                 # [128, 9*64]
OFF_W2V = 882                   # [128, 128] block-diag vals half of W_attn
OFF_W2A = 1010                  # [128, 128] block-diag attn half of W_attn
OFF_WLO = 1138                  # [128, 64] W_enc on bottom rows only
OFF_MASK = 1202                 # [128, 128*16] pixel-major mask
CONSTB_N = OFF_MASK + H * 16


def build_program():
    nc = bacc.Bacc(None, target_bir_lowering=False, debug=False)

    encf_d = nc.dram_tensor("encf", [ENC, ENCF_N], BF16, kind="ExternalInput").ap()
    decp_d = nc.dram_tensor("decp", [DEC, H + 2, W + 2], BF16,
                            kind="ExternalInput").ap()
    fqk_d = nc.dram_tensor("fqk", [NCH, 128, FQ_NR, W], FP8,
                           kind="ExternalInput").ap()
    cq_d = nc.dram_tensor("constq", [128, CONSTQ_N], FP8,
                          kind="ExternalInput").ap()
    cb_d = nc.dram_tensor("constb", [128, CONSTB_N], BF16, kind="ExternalInput").ap()
    cf_d = nc.dram_tensor("constf", [128, 4], FP32, kind="ExternalInput").ap()
    out_d = nc.dram_tensor("out", [ENC, H, W], BF16, kind="ExternalOutput").ap()

    HP, WP = H + 2, W + 2

    with tile.TileContext(nc) as tc, ExitStack() as ctx:
        const = ctx.enter_context(tc.tile_pool(name="const", bufs=1))
        big = ctx.enter_context(tc.tile_pool(name="big", bufs=1))
        fqp = ctx.enter_context(tc.tile_pool(name="fqp", bufs=3))
        sp5p = ctx.enter_context(tc.tile_pool(name="sp5p", bufs=3))
        entp = ctx.enter_context(tc.tile_pool(name="entp", bufs=3))
        catp = ctx.enter_context(tc.tile_pool(name="catp", bufs=2))
        accb = ctx.enter_context(tc.tile_pool(name="accb", bufs=3))
        prodp = ctx.enter_context(tc.tile_pool(name="prodp", bufs=2))
        smal = ctx.enter_context(tc.tile_pool(name="smal", bufs=2))
        outp = ctx.enter_context(tc.tile_pool(name="outp", bufs=1))

        constb = const.tile([128, CONSTB_N], BF16)
        nc.sync.dma_start(constb[:, 0:OFF_MASK], cb_d[:, 0:OFF_MASK])
        nc.sync.dma_start(constb[:, OFF_MASK:], cb_d[:, OFF_MASK:])
        constf = const.tile([128, 4], FP32)
        nc.sync.dma_start(constf[:], cf_d)
        constq = const.tile([128, CONSTQ_N], FP8)
        nc.sync.dma_start(constq[:], cq_d)
        wqk_pair = constq[:, OFF_QKPAIR:OFF_QKPAIR + 256].rearrange(
            "p (t m) -> p t m", t=2)
        wqk_sing = constq[:, OFF_QKSING:OFF_QKSING + 128].rearrange(
            "p (t m) -> p t m", t=2)
        wagg01 = constq[:, OFF_AGG01:OFF_AGG01 + 32].rearrange(
            "p (t m) -> p t m", t=2)
        wagg23 = constq[:, OFF_AGG23:OFF_AGG23 + 32].rearrange(
            "p (t m) -> p t m", t=2)
        wagg4 = constq[0:64, OFF_AGG4:OFF_AGG4 + 16]

        wenc_bd = constb[:, OFF_WENCBD:OFF_WENCBD + 128]
        wdec2 = constb[:, OFF_WDEC2:OFF_WDEC2 + 128]
        wagg5 = constb[:, OFF_WAGG5:OFF_WAGG5 + 50].rearrange(
            "p (g m) -> p g m", g=5)
        convw = constb[:, OFF_CONVW:OFF_CONVW + 576].rearrange(
            "p (n c) -> p n c", n=9)
        w2v = constb[:, OFF_W2V:OFF_W2V + 128]
        w2a = constb[:, OFF_W2A:OFF_W2A + 128]
        wenc_lo = constb[:, OFF_WLO:OFF_WLO + 64]
        maskpm = constb[:, OFF_MASK:OFF_MASK + H * 16].rearrange(
            "p (h n) -> p h n", h=H)
        bsum = constf[:, 0:1]          # b_dec+b_enc, duplicated both halves
        bconv = constf[0:64, 1:2]
        battn2 = constf[:, 2:3]        # b_attn duplicated both halves
        baggb = constf[:, 3:4]         # b_agg replicated on all partitions

        decp = big.tile([DEC, HP, WP], BF16)

        def emit_decp_loads():
            for dli in range(4):
                dr0 = dli * (HP // 4)
                dr1 = HP if dli == 3 else (dli + 1) * (HP // 4)
                nc.sync.dma_start(decp[:, dr0:dr1, :], decp_d[:, dr0:dr1, :])

        # score staging: persistent pair, rows 10:15 stay -100 forever
        sc_sbs = [big.tile([16, RC * W], BF16, tag=f"sc_sb{i}",
                           name=f"sc_sb{i}")
                  for i in range(2)]
        for t in sc_sbs:
            nc.gpsimd.memset(t[:], -100.0 * SA_AGG)

        psco = ctx.enter_context(
            tc.tile_pool(name="psco", bufs=1, space=bass.MemorySpace.PSUM))
        psd = ctx.enter_context(
            tc.tile_pool(name="psd", bufs=1, space=bass.MemorySpace.PSUM))
        psc = ctx.enter_context(
            tc.tile_pool(name="psc", bufs=1, space=bass.MemorySpace.PSUM))
        psf = ctx.enter_context(
            tc.tile_pool(name="psf", bufs=1, space=bass.MemorySpace.PSUM))

        st = {}   # per-chunk front-stage tiles
        acc_st = {}
        vals_st = {}
        attn_st = {}

        def emit_front_dma(ch):
            r0 = ch * RC
            # fqs staging loads first (scores need them immediately); the
            # encoder neighborhood transposes for the MAC have a chunk of
            # slack.
            fqs = fqp.tile([128, FQ_NR, W], FP8, tag="fqs")
            nc.sync.dma_start(fqs[:, 0:FQ_ENCP, :], fqk_d[ch, :, 0:FQ_ENCP, :])
            nc.sync.dma_start(fqs[:, FQ_ENCP:FQ_ENCQ, :],
                              fqk_d[ch, :, FQ_ENCP:FQ_ENCQ, :])
            nc.sync.dma_start(fqs[:, FQ_ENCQ:, :], fqk_d[ch, :, FQ_ENCQ:, :])
            entv = []
            for vi, dc in enumerate((-1, 0, 1)):
                ev = entp.tile([128, RC + 2, ENC], BF16, tag=f"ent{vi}",
                               name=f"ent{vi}")
                nc.sync.dma_start_transpose(
                    ev[:],
                    encf_d[:, ENCF_PAD + dc + (r0 - 1) * W:
                           ENCF_PAD + dc + (r0 + RC + 1) * W])
                entv.append(ev)
            sp5 = sp5p.tile([128, 5, RC, W], FP8, tag="sp5")
            st[ch] = {"sp5": sp5, "entv": entv, "fqs": fqs}

        def plane2(fqs, rl, delta):
            # [128, 2, 4, W] AP: plane 0 = dec rows rl..rl+3, plane 1 = the
            # rows delta*W elements later (the shifted enc copy).
            a = fqs[:, rl:rl + 4, :].unsqueeze(1)
            ap = [list(d) for d in a.ap]
            ap[1] = [delta * W, 2]
            return dataclasses.replace(a, ap=tuple(tuple(d) for d in ap))

        DBLROW = mybir.MatmulPerfMode.DoubleRow

        def emit_front_g(ch, gi):
            # fused q+k DoubleRow fp8 matmuls into PSUM; tanh((q+k)/SW_QK +
            # bsum) -> sp5 planes, for one 4-row group
            fqs, sp5 = st[ch]["fqs"], st[ch]["sp5"]
            rl = gi * 4
            psA = psco.tile([128, 2, 4, W], FP32, tag="psA")
            for half, dr in enumerate((-1, 0)):
                nc.tensor.matmul(psA[:, half], wqk_pair,
                                 plane2(fqs, rl, FQ_ENCP + 1 + dr),
                                 start=True, stop=True, perf_mode=DBLROW)
            nc.scalar.activation(sp5[:, 0:2, rl:rl + 4, :], psA[:],
                                 AF.Tanh, bias=bsum, scale=1.0 / SW_QK)
            psB = psco.tile([128, 2, 4, W], FP32, tag="psB")
            nc.tensor.matmul(psB[:, 0], wqk_pair,
                             plane2(fqs, rl, FQ_ENCP + 2),
                             start=True, stop=True, perf_mode=DBLROW)
            nc.tensor.matmul(psB[:, 1], wqk_pair,
                             plane2(fqs, rl, FQ_ENCQ),
                             start=True, stop=True, perf_mode=DBLROW)
            nc.scalar.activation(sp5[:, 2:4, rl:rl + 4, :], psB[:],
                                 AF.Tanh, bias=bsum, scale=1.0 / SW_QK)
            psC = psco.tile([64, 4, W], FP32, tag="psC")
            nc.tensor.matmul(psC[:], wqk_sing,
                             plane2(fqs, rl, FQ_ENCQ + 1),
                             start=True, stop=True, perf_mode=DBLROW)
            nc.scalar.activation(sp5[0:64, 4, rl:rl + 4, :], psC[:],
                                 AF.Tanh, bias=bsum[0:64], scale=1.0 / SW_QK)

        def emit_attn_tp(b):
            # issue acc -> attn_pc XBAR transpose as soon as block b's MAC is
            # done; finals two chunks later never wait on it.
            acc_blk = acc_st.pop(b)
            attn_pc = catp.tile([128, RC, W], BF16, tag="attn_pc",
                                name="attn_pc")
            nc.sync.dma_start_transpose(
                attn_pc[:], acc_blk[:].rearrange("p h c -> p (h c)"))
            attn_st[b] = attn_pc

        def emit_attn_hc(ch, hc):
            sp5 = st[ch]["sp5"]
            sc_sb = sc_sbs[ch % 2]
            dps = psd.tile([16, 4 * W], FP32, tag="dps")
            rl = hc * 4
            nc.tensor.matmul(dps[:], wagg01, sp5[:, 0:2, rl:rl + 4, :],
                             start=True, stop=False, perf_mode=DBLROW)
            nc.tensor.matmul(dps[:], wagg23, sp5[:, 2:4, rl:rl + 4, :],
                             start=False, stop=False, perf_mode=DBLROW)
            nc.tensor.matmul(
                dps[:], wagg4, sp5[0:64, 4, rl:rl + 4, :],
                start=False, stop=True)
            nc.scalar.activation(
                sc_sb[0:10, hc * 4 * W:(hc + 1) * 4 * W], dps[0:10], AF.Copy)

        def emit_attn_post(ch):
            r0 = ch * RC
            lr0 = (ch % 2) * RC
            entv = st[ch]["entv"]
            if ch % 2 == 0:
                acc_st[ch // 2] = accb.tile([128, RC2, ENC], BF16,
                                            tag="acc_blk", name="acc_blk")
            acc_blk = acc_st[ch // 2]
            sc_sb = sc_sbs[ch % 2]
            # transpose + softmax
            e_pm = smal.tile([128, RC, 16], BF16, tag="e_pm")
            nc.sync.dma_start_transpose(e_pm[:], sc_sb[:])
            nc.scalar.activation(
                e_pm[:].rearrange("p r n -> p (r n)"),
                e_pm[:].rearrange("p r n -> p (r n)"), AF.Exp, bias=baggb,
                scale=1.0 / SA_AGG)
            nc.vector.tensor_tensor(
                e_pm[:], e_pm[:], maskpm[:, r0:r0 + RC, :], ALU.mult)
            zs = smal.tile([128, RC], FP32, tag="zs")
            nc.vector.tensor_reduce(
                out=zs[:], in_=e_pm[:], axis=mybir.AxisListType.X, op=ALU.add)
            zr = smal.tile([128, RC], FP32, tag="zr")
            nc.vector.reciprocal(zr[:], zs[:])
            e_pm2 = smal.tile([128, RC, 16, 2], BF16, tag="e_pm2")
            nc.vector.tensor_tensor(
                e_pm2[:],
                e_pm[:].unsqueeze(3).broadcast_to([128, RC, 16, 2]),
                zr[:].unsqueeze(2).unsqueeze(3).broadcast_to([128, RC, 16, 2]),
                ALU.mult)
            # MAC
            a4 = acc_blk.rearrange("p h (a b) -> p h a b", b=2)
            macord = [3] + [j for j in range(9) if j != 3]
            for j in macord:
                dr, dc = OFFS[NMAP[j]]
                src = entv[dc + 1][:, 1 + dr:1 + dr + RC, :].rearrange(
                    "p r (a b) -> p r a b", b=2)
                wsl = e_pm2[:, :, j:j + 1, :].broadcast_to(
                    [128, RC, ENC // 2, 2])
                if j == 3:
                    nc.vector.tensor_tensor(
                        a4[:, lr0:lr0 + RC], wsl, src, ALU.mult)
                else:
                    prod = prodp.tile([128, RC, ENC // 2, 2], BF16,
                                      tag="prod")
                    nc.vector.tensor_tensor(prod[:], wsl, src, ALU.mult)
                    nc.vector.tensor_tensor(
                        a4[:, lr0:lr0 + RC], a4[:, lr0:lr0 + RC], prod[:],
                        ALU.add)

        def emit_conv_w(ch, wi):
            r0 = ch * RC
            if ch % 2 == 0 and wi == 0:
                vals_st[ch // 2] = catp.tile([128, RC, W], BF16,
                                             tag="vals_pc", name="vals_pc")
            vals_pc = vals_st[ch // 2]
            cp = psc.tile([ENC, 4, W], FP32, tag="cp")
            wr0 = r0 + wi * 4
            for n, (dr, dc) in enumerate(OFFS):
                nc.tensor.matmul(
                    cp[:], convw[:, n, :],
                    decp[:, 1 + wr0 + dr:1 + wr0 + 4 + dr,
                         1 + dc:1 + W + dc],
                    start=(n == 0), stop=(n == 8))
            lh2 = ((ch % 2) * RC + wi * 4) // 2
            nc.scalar.activation(
                vals_pc[0:64, lh2:lh2 + 2, :], cp[:, 0::2, :],
                AF.Identity, bias=bconv)
            nc.scalar.activation(
                vals_pc[64:128, lh2:lh2 + 2, :], cp[:, 1::2, :],
                AF.Identity, bias=bconv)

        def emit_final(b):
            vals_pc = vals_st.pop(b)
            attn_pc = attn_st.pop(b)
            b0 = b * RC2
            outsb = outp.tile([ENC, RC2, W], BF16, tag="outsb")
            for wi in range(RC // 4):
                fp = psf.tile([128, 4, W], FP32, tag="fp")
                fpf = fp[:].rearrange("c r w -> c (r w)")
                nc.tensor.matmul(
                    fpf, w2v, vals_pc[:, wi * 4:(wi + 1) * 4, :],
                    start=True, stop=False)
                nc.tensor.matmul(
                    fpf, w2a, attn_pc[:, wi * 4:(wi + 1) * 4, :],
                    start=False, stop=True)
                ob0 = wi * 8
                tll = smal.tile([128, 4, W], BF16, tag="tll")
                nc.scalar.activation(tll[:], fp[:], AF.Identity, bias=battn2)
                nc.vector.scalar_tensor_tensor(
                    outsb[:, ob0:ob0 + 8:2, :], tll[0:64], 0.2,
                    tll[0:64], ALU.mult, ALU.max)
                nc.vector.scalar_tensor_tensor(
                    outsb[:, ob0 + 1:ob0 + 8:2, :], tll[64:128], 0.2,
                    tll[64:128], ALU.mult, ALU.max)
            nc.sync.dma_start(out_d[:, b0:b0 + RC2, :], outsb[:])

        # Fine-grained interleave: per 4-row group, emit next chunk's
        # q+k+tanh, this chunk's dots, and the conv window from 2 chunks
        # back — so each in-order engine queue alternates independent work
        # and ring-1 PSUM reuse never stalls the head of the queue.
        # Finals fire at chunks 3/5/7; tail is conv6, conv7, final3.
        finals_at = {3: 0, 5: 1, 7: 2}
        emit_front_dma(0)
        emit_front_dma(1)
        emit_decp_loads()
        for gi in range(4):
            emit_front_g(0, gi)
        for gi in range(4):
            emit_front_g(1, gi)
        for ch in range(NCH):
            if ch + 2 < NCH:
                emit_front_dma(ch + 2)
            for gi in range(4):
                emit_attn_hc(ch, gi)
                if ch >= 2:
                    emit_conv_w(ch - 2, gi)
                if ch + 2 < NCH:
                    emit_front_g(ch + 2, gi)
            emit_attn_post(ch)
            if ch % 2 == 1:
                emit_attn_tp(ch // 2)
            if ch in finals_at:
                emit_final(finals_at[ch])
        for wi in range(4):
            emit_conv_w(6, wi)
        for wi in range(4):
            emit_conv_w(7, wi)
        emit_final(3)

    nc.compile()
    return nc


_PROG = None
_RUN_KWARGS = {}
_LAST_RESULT = None


def _get_prog():
    global _PROG
    if _PROG is None:
        _PROG = build_program()
    return _PROG


def _make_mask_pm():
    """[W(part), H, 16] validity mask in NMAP column order."""
    m = np.zeros((W, H, 16), dtype=np.float32)
    for j, n in enumerate(NMAP):
        dr, dc = OFFS[n]
        rv = np.arange(H) + dr
        cv = np.arange(W) + dc
        m[:, :, j] = (((cv >= 0) & (cv < W))[:, None]
                      & ((rv >= 0) & (rv < H))[None, :]).astype(np.float32)
    return m


def _pack_constb(W_dec, W_enc, W_agg, conv_w, W_attn):
    bf = ml_dtypes.bfloat16
    cb = np.zeros((128, CONSTB_N), dtype=np.float32)
    we = np.asarray(W_enc, np.float32)
    cb[0:64, OFF_WENCBD:OFF_WENCBD + 64] = we
    cb[64:128, OFF_WENCBD + 64:OFF_WENCBD + 128] = we
    cb[64:128, OFF_WLO:OFF_WLO + 64] = we
    wd = np.asarray(W_dec, np.float32)
    cb[:, OFF_WDEC2:OFF_WDEC2 + 64] = wd
    cb[:, OFF_WDEC2 + 64:OFF_WDEC2 + 128] = wd
    wa = np.asarray(W_agg, np.float32)[:, 0]
    w5 = np.zeros((128, 5, 10), dtype=np.float32)
    for g in range(4):  # groups 0-2 pairs, group 3 stacked singles
        w5[0:64, g, 2 * g] = wa
        w5[64:128, g, 2 * g + 1] = wa
    w5[0:64, 4, 8] = wa
    cb[:, OFF_WAGG5:OFF_WAGG5 + 50] = w5.reshape(128, 50)
    cw = np.asarray(conv_w, np.float32).reshape(9, DEC, ENC).transpose(1, 0, 2)
    cb[:, OFF_CONVW:OFF_CONVW + 576] = cw.reshape(128, 576)
    wat = np.asarray(W_attn, np.float32)
    for par in range(2):  # block-diag: row parity stays separated
        sl = slice(par * 64, par * 64 + 64)
        cb[sl, OFF_W2V + par * 64:OFF_W2V + par * 64 + 64] = wat[0:64]
        cb[sl, OFF_W2A + par * 64:OFF_W2A + par * 64 + 64] = wat[64:128]
    cb[:, OFF_MASK:OFF_MASK + H * 16] = _make_mask_pm().reshape(128, H * 16)
    return cb.astype(bf)


def _pack_fqk(dec, enc):
    """[NCH, 128, FQ_NR, W] staging: dec rows | encP stacked | encQ stacked.

    dec: [128, H, W]; enc: [64, H, W].
    encP row x: top half enc[c, x, w-1], bottom enc[c, x, w].
    encQ row y: top half enc[c, y, w+1], bottom enc[c, y+1, w+1].
    Out-of-range rows/cols are zero.
    """
    dec = np.asarray(dec, np.float32)
    enc = np.asarray(enc, np.float32)
    encm = np.zeros((ENC, H, W), np.float32)   # enc shifted to w-1 frame
    encm[:, :, 1:] = enc[:, :, :-1]
    encs = np.zeros((ENC, H, W), np.float32)   # enc shifted to w+1 frame
    encs[:, :, :-1] = enc[:, :, 1:]

    def padr(a):  # rows -1 and H become zero; row x lives at index x+1
        return np.pad(a, ((0, 0), (1, 1), (0, 0)))

    encm_p, enc_p, encs_p = padr(encm), padr(enc), padr(encs)
    fqk = np.zeros((NCH, 128, FQ_NR, W), np.float32)
    for ch in range(NCH):
        r0 = ch * RC
        fqk[ch, :, FQ_DEC:FQ_DEC + RC] = dec[:, r0:r0 + RC]
        fqk[ch, 0:64, FQ_ENCP:FQ_ENCQ] = encm_p[:, r0:r0 + RC + 2]
        fqk[ch, 64:128, FQ_ENCP:FQ_ENCQ] = enc_p[:, r0:r0 + RC + 2]
        fqk[ch, 0:64, FQ_ENCQ:FQ_NR] = encs_p[:, r0:r0 + RC + 1]
        fqk[ch, 64:128, FQ_ENCQ:FQ_NR] = encs_p[:, r0 + 1:r0 + RC + 2]
    return fqk.astype(ml_dtypes.float8_e4m3)


def _pack_constq(W_dec, W_enc, W_agg):
    cq = np.zeros((128, CONSTQ_N), np.float32)
    wd = np.asarray(W_dec, np.float32) * SW_QK   # [128, 64]
    we = np.asarray(W_enc, np.float32) * SW_QK   # [64, 64]
    cq[:, OFF_QKPAIR + 0:OFF_QKPAIR + 64] = wd
    cq[:, OFF_QKPAIR + 64:OFF_QKPAIR + 128] = wd
    cq[0:64, OFF_QKPAIR + 128:OFF_QKPAIR + 192] = we
    cq[64:128, OFF_QKPAIR + 192:OFF_QKPAIR + 256] = we
    cq[:, OFF_QKSING:OFF_QKSING + 64] = wd
    cq[64:128, OFF_QKSING + 64:OFF_QKSING + 128] = we
    wa = np.asarray(W_agg, np.float32)[:, 0] * SA_AGG
    w5 = np.zeros((128, 5, 16), np.float32)
    for g in range(4):
        w5[0:64, g, 2 * g] = wa
        w5[64:128, g, 2 * g + 1] = wa
    w5[0:64, 4, 8] = wa
    cq[:, OFF_AGG01:OFF_AGG01 + 16] = w5[:, 0]
    cq[:, OFF_AGG01 + 16:OFF_AGG01 + 32] = w5[:, 1]
    cq[:, OFF_AGG23:OFF_AGG23 + 16] = w5[:, 2]
    cq[:, OFF_AGG23 + 16:OFF_AGG23 + 32] = w5[:, 3]
    cq[0:64, OFF_AGG4:OFF_AGG4 + 16] = w5[0:64, 4]
    return np.clip(cq, -240, 240).astype(ml_dtypes.float8_e4m3)


def kernel(encoder_features, decoder_features, W_enc, b_enc, W_dec, b_dec,
           W_agg, b_agg, W_attn, b_attn, conv_w, conv_b):
    bf = ml_dtypes.bfloat16
    nc = _get_prog()

    cf = np.zeros((128, 4), dtype=np.float32)
    bs = np.asarray(b_dec, np.float32) + np.asarray(b_enc, np.float32)
    cf[0:64, 0] = bs
    cf[64:128, 0] = bs
    cf[0:64, 1] = np.asarray(conv_b, np.float32)
    cf[0:64, 2] = np.asarray(b_attn, np.float32)
    cf[64:128, 2] = np.asarray(b_attn, np.float32)
    cf[:, 3] = float(np.asarray(b_agg).reshape(-1)[0])

    shared = {
        "constb": _pack_constb(W_dec, W_enc, W_agg, conv_w, W_attn),
        "constf": cf,
        "constq": _pack_constq(W_dec, W_enc, W_agg),
    }
    enc_all = np.asarray(encoder_features, np.float32).astype(bf)
    dec_all = np.asarray(decoder_features, np.float32).astype(bf)
    in_maps = []
    for c in range(N_CORES):
        encf = np.zeros((ENC, ENCF_N), dtype=bf)
        encf[:, ENCF_PAD:ENCF_PAD + H * W] = enc_all[c].reshape(ENC, H * W)
        decp = np.zeros((DEC, H + 2, W + 2), dtype=bf)
        decp[:, 1:H + 1, 1:W + 1] = dec_all[c]
        m = dict(shared)
        m["encf"] = encf
        m["decp"] = decp
        m["fqk"] = _pack_fqk(dec_all[c].astype(np.float32),
                             enc_all[c].astype(np.float32))
        in_maps.append(m)

    res = run_bass_kernel_spmd(nc, in_maps, list(range(N_CORES)),
                               **_RUN_KWARGS)
    global _LAST_RESULT
    _LAST_RESULT = res
    out = np.stack(
        [np.asarray(res.results[c]["out"], np.float32) for c in range(N_CORES)])
    return out
